# revision 9
# baseline (speedup 1.0000x reference)
"""AdaptiveGCN kernel for TRN2 (8 NeuronCores, SPMD).

Reference math (B=4, D=128, N=512):
    A = W1 @ x[b]                  # [D, N]
    C = W2 @ x[b] + b[:, None]     # [D, N]
    pre[d, i, j] = A[d, j] + (C - A)[d, i]
    out[d, i] = max_j relu(pre[d, i, j])

Since (C - A)[d, i] is constant in j and relu/max commute (both monotone),
    out[d, i] = relu(max_j A[d, j] + ((W2 - W1) @ x[b])[d, i] + b[d])
The [N, N] pairwise grid never materializes.

Sharding: one batch per core (cores 4..7 duplicate batches 0..3 and are
ignored on gather) — no cross-core communication needed.

Layout/perf notes:
- bf16 compute (inputs pre-cast and weights pre-transposed on the host;
  W1^T and (W2-W1)^T ship in one [128, 256] tile, x in [128, 512]).
  Power-of-two per-partition byte counts avoid HWDGE straggler chunks.
- Output is computed and DMA'd in two column halves so the second half's
  vector op overlaps the first half's writeback; bf16 out, upcast on host.
- Overall rel-err ~2e-3, well inside the 2e-2 gate.
"""

import numpy as np
import ml_dtypes

import concourse.bacc as bacc
import concourse.tile as tile
from concourse import mybir
from concourse.bass_utils import run_bass_kernel_spmd

F32 = mybir.dt.float32
BF16 = mybir.dt.bfloat16
B, D, N = 4, 128, 512
H = N // 2
N_CORES = 8

_NC_CACHE = None


def _build():
    nc = bacc.Bacc(
        "TRN2", target_bir_lowering=False, debug=False, num_devices=N_CORES
    )
    wb = nc.declare_dram_parameter("wb", [D, 2 * D], BF16, isOutput=False)
    x = nc.declare_dram_parameter("x", [D, N], BF16, isOutput=False)
    bv = nc.declare_dram_parameter("bv", [D, 1], F32, isOutput=False)
    out = nc.declare_dram_parameter("out", [D, N], BF16, isOutput=True)

    with tile.TileContext(nc) as tc:
        with (
            tc.tile_pool(name="sb", bufs=1) as sb,
            tc.tile_pool(name="ps", bufs=1, space="PSUM") as ps,
        ):
            wb_t = sb.tile([D, 2 * D], BF16)
            nc.sync.dma_start(out=wb_t, in_=wb[:, :])
            x_t = sb.tile([D, N], BF16)
            nc.sync.dma_start(out=x_t, in_=x[:, :])
            b_t = sb.tile([D, 1], F32)
            nc.sync.dma_start(out=b_t, in_=bv[:, :])
            w1T_v = wb_t[:, 0:D]
            wdT_v = wb_t[:, D : 2 * D]

            # A = W1 @ x -> [D, N] f32 in PSUM (one bank)
            p_a = ps.tile([D, N], F32)
            nc.tensor.matmul(p_a, w1T_v, x_t, start=True, stop=True)

            # V = (W2 - W1) @ x -> [D, N]
            p_v = ps.tile([D, N], F32)
            nc.tensor.matmul(p_v, wdT_v, x_t, start=True, stop=True)

            # amax[d] = max_j A[d, j]; tvec = amax + bias
            amax = sb.tile([D, 1], F32)
            nc.vector.reduce_max(out=amax, in_=p_a, axis=mybir.AxisListType.X)
            tvec = sb.tile([D, 1], F32)
            nc.vector.tensor_add(tvec, amax, b_t)

            # out = relu(V + tvec) = (V + tvec) max 0, fused on DVE.
            # Two column halves so half-1's DMA overlaps half-2's compute.
            o_t = sb.tile([D, N], BF16)
            for h in range(2):
                cols = slice(h * H, (h + 1) * H)
                nc.vector.tensor_scalar(
                    out=o_t[:, cols],
                    in0=p_v[:, cols],
                    scalar1=tvec,
                    scalar2=0.0,
                    op0=mybir.AluOpType.add,
                    op1=mybir.AluOpType.max,
                )
                nc.sync.dma_start(out=out[:, cols], in_=o_t[:, cols])
    nc.finalize()
    return nc


def _in_maps(x, W1, W2, b):
    bf = ml_dtypes.bfloat16
    x = np.asarray(x, dtype=np.float32)
    W1 = np.asarray(W1, dtype=np.float32)
    W2 = np.asarray(W2, dtype=np.float32)
    b = np.asarray(b, dtype=np.float32)
    wb = np.ascontiguousarray(
        np.concatenate([W1.T, (W2 - W1).T], axis=1)
    ).astype(bf)
    bvec = np.ascontiguousarray(b[:, None])
    xs = [np.ascontiguousarray(x[c % B]).astype(bf) for c in range(N_CORES)]
    return [
        {"wb": wb, "x": xs[c], "bv": bvec} for c in range(N_CORES)
    ]


def kernel_raw(x, W1, W2, b, **run_kwargs):
    """Run the SPMD kernel; returns (full_output, BassKernelResults)."""
    global _NC_CACHE
    if _NC_CACHE is None:
        _NC_CACHE = _build()
    res = run_bass_kernel_spmd(
        _NC_CACHE, _in_maps(x, W1, W2, b), core_ids=list(range(N_CORES)),
        **run_kwargs,
    )
    out = np.stack(
        [res.results[c]["out"].astype(np.float32) for c in range(B)], axis=0
    )
    return out, res


def kernel(x, W1, W2, b):
    return kernel_raw(x, W1, W2, b)[0]
